# revision 18
# baseline (speedup 1.0000x reference)
"""Multi-head attention (dense transformer block) on 8 Trainium2 NeuronCores.

Sharding: batch (2) x query-row-block (4) -> 8 cores; each core computes the
full attention output for its 512 query rows of one batch.

v3 pipeline (per core):
  - Q/K projections: fp8e4 DoubleRow matmuls (0.5 cyc/row), outputs stored
    fp8 in head-pair tiles (heads 2m / 2m+1 on partition halves).
  - scores: fp8 DoubleRow with a shared zero block-1 (custom-stride APs), so
    the 64-deep head contraction runs at 0.5 cyc/row.
  - exp on ACT (psum f32 -> fp16, scale folds the 1/sqrt(HD)); the 0/1 mask
    is applied as an in-place fp16 DVE multiply (exact zeros).
  - V projection in bf16 (fp8 here dominates the error budget); V stored
    fp16 with a ones column for softmax denominators.
  - p@[V|1] in fp16; denominators -> reciprocal -> broadcast -> scale.
  - output projection bf16.
  - K bias is dropped entirely: it shifts scores per-query only and cancels
    in softmax.
Scale bookkeeping: W1/W2 sent x16 (fp8 subnormal headroom), rescaled at PSUM
evacuation; q kept at 4x the 1/8-scaled value, Exp applies scale=0.25.
"""

import numpy as np
import ml_dtypes
import bass_rust
import concourse.bass as bass
import concourse.mybir as mybir
from concourse import bacc
from concourse.tile import TileContext
from concourse.bass_utils import run_bass_kernel_spmd

B, S, D, H, HD = 2, 2048, 512, 8, 64
P = 128
NCORES = 8
RPB = 4              # q-row blocks per batch
QB = S // RPB        # 512 query rows per core
NDC = D // P         # 4 chunks of the model dim (contraction subtiles)
NKT = S // P         # 16 key tiles
NG = NKT // 2        # 8 key-tile pairs (one scores psum tile each)
VW = HD + 1          # 65 columns per head in vext (64 v + 1 ones)
KZ = NKT * P         # zero-block column offset in kp tiles
QZ = QB              # zero-block column offset in qp tiles

f32 = mybir.dt.float32
f32r = mybir.dt.float32r
f16 = mybir.dt.float16
f8 = mybir.dt.float8e4
bf16 = mybir.dt.bfloat16
u32 = mybir.dt.uint32
Alu = mybir.AluOpType
ActF = mybir.ActivationFunctionType
DR = mybir.MatmulPerfMode.DoubleRow

FP8 = ml_dtypes.float8_e4m3fn
BF16 = ml_dtypes.bfloat16


def _dr_ap(tile, prow, nrow, dims, coloff):
    """Custom strided AP into `tile` starting at partition `prow`, column
    `coloff`, with explicit free dims `dims` ([[stride, count], ...])."""
    a = tile[:].copy()
    rs = a.ap[0][0]
    a.ap = bass_rust.VecI64Pair([[rs, nrow]] + dims)
    a.offset = prow * rs + coloff
    return a


def _build_nc(repeats=1):
    nc = bacc.Bacc("TRN2", target_bir_lowering=False, debug=False,
                   num_devices=NCORES)
    xqd = nc.dram_tensor("xqd", [P, NDC * QB], f8, kind="ExternalInput")
    xkd = nc.dram_tensor("xkd", [P, NDC * S], f8, kind="ExternalInput")
    # xv layout: [c, kb(2), s(4), key%1024] for key-range-split DMA
    xvd = nc.dram_tensor("xvd", [P, NDC * S], bf16, kind="ExternalInput")
    w1d = nc.dram_tensor("w1d", [P, NDC * D], f8, kind="ExternalInput")
    w2d = nc.dram_tensor("w2d", [P, NDC * D], f8, kind="ExternalInput")
    w3d = nc.dram_tensor("w3d", [P, NDC * D], bf16, kind="ExternalInput")
    wod = nc.dram_tensor("wod", [P, NDC * D], bf16, kind="ExternalInput")
    # mask [c, g(8), j(2), q] in {0,1} fp16
    mkd = nc.dram_tensor("mkd", [P, NKT * QB], f16, kind="ExternalInput")
    b1d = nc.dram_tensor("b1d", [P, NDC], f32, kind="ExternalInput")
    bod = nc.dram_tensor("bod", [P, D], f32, kind="ExternalInput")
    y = nc.dram_tensor("y", [QB, D], f32, kind="ExternalOutput")

    with TileContext(nc) as tc, nc.allow_low_precision("fp8 attention"):
        with (
            tc.tile_pool(name="persist", bufs=1) as pp,
            tc.tile_pool(name="small", bufs=1) as sp,
            tc.tile_pool(name="psp", bufs=2, space="PSUM") as psp,
            tc.tile_pool(name="ps_s", bufs=2, space="PSUM") as ps_s,
            tc.tile_pool(name="ps_a", bufs=2, space="PSUM") as ps_a,
            tc.tile_pool(name="pex", bufs=18) as pex,
            tc.tile_pool(name="asm", bufs=2) as asm,
            tc.tile_pool(name="yp", bufs=4) as yp,
        ):
            # ---- persistent SBUF tiles ----
            # kp[m]: heads 2m (part 0-63) / 2m+1 (64-127); cols t*128+k,
            # shared zero block at [*, KZ:KZ+128]
            kp = [pp.tile([P, KZ + P], f8, tag=f"kp{m}", name=f"kp{m}")
                  for m in range(NDC)]
            # qp[m]: cols 0..511 = q data, 512..767 shared zero
            qp = [pp.tile([P, QB + 256], f8, tag=f"qp{m}", name=f"qp{m}")
                  for m in range(NDC)]
            # vext[g]: [128, h*2*VW + j*VW + e] fp16, ones at e=64
            vext = [pp.tile([P, H * 2 * VW], f16, tag=f"vx{g}",
                            name=f"vx{g}") for g in range(NG)]
            mk = pp.tile([P, NKT * QB], f16, tag="mk", name="mk")
            wosb = pp.tile([P, NDC * D], bf16, tag="wosb", name="wosb")
            outT = [pp.tile([P, QB], bf16, tag=f"outT{m}", name=f"outT{m}")
                    for m in range(NDC)]

            xq = sp.tile([P, NDC * QB], f8, tag="xq", name="xq")
            xk = sp.tile([P, NDC * S], f8, tag="xk", name="xk")
            xv = sp.tile([P, NDC * S], bf16, tag="xv", name="xv")
            w1 = sp.tile([P, NDC * D], f8, tag="w1", name="w1")
            w2 = sp.tile([P, NDC * D], f8, tag="w2", name="w2")
            w3 = sp.tile([P, NDC * D], bf16, tag="w3", name="w3")
            b1c = sp.tile([P, NDC], f32, tag="b1c", name="b1c")
            bob = sp.tile([P, D], f32, tag="bob", name="bob")
            ones_r = sp.tile([1, P], f32r, tag="ones_r", name="ones_r")
            nc.vector._memset_packed(ones_r[:].bitcast(u32), 0x3F800000)

            # ---- DMA (two rings: sync=SP, gpsimd=Pool) ----
            HX = NDC * S // 2
            nc.gpsimd.dma_start(w1[:], w1d[:])
            nc.gpsimd.dma_start(xk[:, HX:], xkd[:, HX:])
            nc.gpsimd.dma_start(w2[:], w2d[:])

            # zero blocks for the DoubleRow score trick; V ones columns
            for m in range(NDC):
                nc.gpsimd.memset(kp[m][:, KZ:KZ + P], 0.0)
                nc.gpsimd.memset(qp[m][:, QZ:QZ + 256], 0.0)
            for g in range(NG):
                vv = vext[g][:].rearrange("p (h j e) -> p h j e", j=2, e=VW)
                nc.gpsimd.memset(vv[:, :, :, HD:VW], 1.0)

            nc.gpsimd.dma_start(w3[:], w3d[:])
            HM = NKT * QB // 2
            nc.sync.dma_start(xq[:], xqd[:])
            nc.sync.dma_start(b1c[:], b1d[:])
            nc.sync.dma_start(xk[:, 0:HX], xkd[:, 0:HX])
            nc.sync.dma_start(mk[:, 0:HM], mkd[:, 0:HM])
            nc.sync.dma_start(xv[:, 0:HX], xvd[:, 0:HX])
            nc.sync.dma_start(mk[:, HM:], mkd[:, HM:])
            nc.sync.dma_start(xv[:, HX:], xvd[:, HX:])
            nc.sync.dma_start(bob[:], bod[:])
            nc.sync.dma_start(wosb[:], wod[:])

            xqv = xq[:].rearrange("p (s q) -> p s q", s=NDC)
            xkv = xk[:].rearrange("p (s k) -> p s k", s=NDC)
            # xv view: [c, kb, s, kk]
            xvv = xv[:].rearrange("p (kb s k) -> p kb s k", kb=2, s=NDC)
            w1v = w1[:].rearrange("p (s d) -> p s d", s=NDC)
            w2v = w2[:].rearrange("p (s d) -> p s d", s=NDC)
            w3v = w3[:].rearrange("p (s d) -> p s d", s=NDC)

            def dr_pair_mm(ps_region, stat3, mov3):
                """512-deep contraction (2 DoubleRow pair-calls) into a
                256-col psum region; completes the group start..stop."""
                for pr in range(2):
                    nc.tensor.matmul(ps_region, stat3(pr), mov3(pr),
                                     start=(pr == 0), stop=(pr == 1),
                                     perf_mode=DR)

            # ==== Q projection ====
            def q_proj(ms):
                for m in ms:
                    ps = psp.tile([P, QB], f32, tag="ps", name="psq")
                    for qh in range(2):
                        dr_pair_mm(
                            ps[:, qh * 256:(qh + 1) * 256],
                            lambda pr: w1v[:, 2 * pr:2 * pr + 2,
                                           m * P:(m + 1) * P],
                            lambda pr: xqv[:, 2 * pr:2 * pr + 2,
                                           qh * 256:(qh + 1) * 256])
                    nc.vector.tensor_scalar(
                        qp[m][:, 0:QB], ps[:],
                        1.0 / 32.0, b1c[:, m:m + 1], Alu.mult, Alu.add)

            # ==== K projection (no bias: cancels in softmax) ====
            # evacs for m>=1 run on Pool: the DVE queue is ACT-gated by the
            # mask multiplies, which would delay these (and the psum ring)
            # by a full exp period per head.
            def k_proj(m):
                for kb in range(NDC):
                    ps = psp.tile([P, QB], f32, tag="ps", name="psk")
                    for qh in range(2):
                        c0 = kb * QB + qh * 256
                        dr_pair_mm(
                            ps[:, qh * 256:(qh + 1) * 256],
                            lambda pr: w2v[:, 2 * pr:2 * pr + 2,
                                           m * P:(m + 1) * P],
                            lambda pr: xkv[:, 2 * pr:2 * pr + 2,
                                           c0:c0 + 256])
                    nc.vector.tensor_scalar(
                        kp[m][:, kb * QB:(kb + 1) * QB], ps[:],
                        1.0 / 16.0, None, Alu.mult)

            # ==== scores + exp + mask ====
            def scores_head(h, embeds=None):
                m, hp = divmod(h, 2)
                base = hp * HD
                pes = []
                for g in range(NG):
                    ss = ps_s.tile([P, 2 * QB], f32, tag="ss", name="ss")
                    for j in range(2):
                        t = 2 * g + j
                        st = _dr_ap(kp[m], base, HD,
                                    [[KZ - t * P, 2], [1, P]], t * P)
                        for qh in range(2):
                            mv = _dr_ap(qp[m], base, HD,
                                        [[QZ - qh * 256, 2], [1, 256]],
                                        qh * 256)
                            nc.tensor.matmul(
                                ss[:, j * QB + qh * 256:
                                   j * QB + (qh + 1) * 256],
                                st, mv, start=True, stop=True, perf_mode=DR)
                    pe = pex.tile([P, 2 * QB], f16, tag="pe", name="pe",
                                  bufs=26)
                    nc.scalar.activation(pe[:], ss[:], ActF.Exp, scale=0.25)
                    # mask: in-place multiply by {0,1} fp16. Late heads go to
                    # Pool so tail reciprocals are not stuck behind the
                    # ACT-gated mult stream in the in-order DVE queue.
                    meng = nc.vector if h == 0 else nc.gpsimd
                    meng.tensor_tensor(
                        pe[:], pe[:], mk[:, g * 2 * QB:(g + 1) * 2 * QB],
                        op=Alu.mult)
                    pes.append(pe)
                    if embeds and g in embeds:
                        embeds[g]()  # e.g. a reciprocal, slotted into the
                        #              ACT-gated gaps of the DVE stream
                return pes

            # ==== V projection (bf16) ====
            def v_proj(t0, t1):
                for t in range(t0, t1):
                    kb, kk = divmod(t, 8)
                    pool = psp if t % 2 == 0 else ps_a
                    ps = pool.tile([P, D], f32, tag="ps" if t % 2 == 0
                                   else "po", name="psv")
                    for s in range(NDC):
                        nc.tensor.matmul(
                            ps[:],
                            xvv[:, kb, s, kk * P:(kk + 1) * P],
                            w3v[:, s, :],
                            start=(s == 0), stop=(s == NDC - 1))
                    g, j = divmod(t, 2)
                    vv = vext[g][:].rearrange("p (h j e) -> p h j e",
                                              j=2, e=VW)
                    nc.vector.tensor_copy(
                        vv[:, :, j:j + 1, 0:HD],
                        ps[:].rearrange("p (h o e) -> p h o e", o=1, e=HD))

            rhs = {}
            pos = {}

            def pv_mm(h, pes):
                """p @ [V|1] for head h (fp16)."""
                po = ps_a.tile([VW, QB], f32, tag="po", name="po")
                for t in range(NKT):
                    g, j = divmod(t, 2)
                    nc.tensor.matmul(
                        po[:],
                        vext[g][:, h * 2 * VW + j * VW:
                                h * 2 * VW + (j + 1) * VW],
                        pes[g][:, j * QB:(j + 1) * QB],
                        start=(t == 0), stop=(t == NKT - 1))
                pos[h] = po

            def rec(h):
                """reciprocal of head h's softmax denominator (DVE)."""
                rh = asm.tile([1, QB], f32, tag="rh", name="rh", bufs=2)
                nc.vector.reciprocal(rh[:], pos[h][HD:VW, :])
                rhs[h] = rh

            def pv_norm(h):
                rb = asm.tile([HD, QB], f32, tag="rb", name="rb", bufs=2)
                nc.gpsimd.partition_broadcast(rb[:], rhs.pop(h)[:])
                m, hp = divmod(h, 2)
                nc.vector.tensor_tensor(
                    outT[m][hp * HD:(hp + 1) * HD, :],
                    pos.pop(h)[0:HD, :], rb[:], op=Alu.mult)

            # ---- emission schedule (ACT = bottleneck; keep it fed).
            # Reciprocals are embedded into later heads' scores streams so
            # the in-order DVE reaches them in the ACT-gated gaps between
            # mask multiplies.
            q_proj([0])
            k_proj(0)
            pes = {0: scores_head(0)}
            q_proj([1, 2, 3])
            k_proj(1)
            pes[1] = scores_head(1)
            k_proj(2)
            k_proj(3)
            v_proj(0, 8)
            pes[2] = scores_head(2)
            v_proj(8, 16)
            pes[3] = scores_head(3)
            pv_mm(0, pes.pop(0))
            pes[4] = scores_head(4, {0: lambda: rec(0)})
            pv_mm(1, pes.pop(1))
            pv_norm(0)
            pes[5] = scores_head(5, {0: lambda: rec(1)})
            pv_mm(2, pes.pop(2))
            pv_norm(1)
            pes[6] = scores_head(6, {0: lambda: rec(2)})
            pv_mm(3, pes.pop(3))
            pv_norm(2)
            pv_mm(4, pes.pop(4))
            rec(3)
            pv_norm(3)
            pv_mm(5, pes.pop(5))
            rec(4)
            pv_norm(4)
            rec(5)
            pes[7] = scores_head(7)
            pv_mm(6, pes.pop(6))
            pv_norm(5)

            # pre-accumulate the first 3 outT chunks of the output
            # projection for q-tiles 0/1 while the tail drains
            yps = {}

            def yp_pre(qt, pool, tag):
                ps = pool.tile([P, D], f32, tag=tag, name="psy")
                for m in range(3):
                    nc.tensor.matmul(
                        ps[:], outT[m][:, qt * P:(qt + 1) * P],
                        wosb[:, m * D:(m + 1) * D],
                        start=(m == 0), stop=False)
                yps[qt] = ps

            def yp_fin(qt):
                ps = yps[qt]
                nc.tensor.matmul(
                    ps[:], outT[3][:, qt * P:(qt + 1) * P],
                    wosb[:, 3 * D:4 * D], start=False, stop=True)
                ysb = yp.tile([P, D], f32, tag="ysb", name="ysb", bufs=4)
                nc.vector.tensor_tensor(ysb[:], ps[:], bob[:], op=Alu.add)
                nc.sync.dma_start(y[qt * P:(qt + 1) * P, :], ysb[:])

            yp_pre(0, psp, "ps")
            yp_pre(1, psp, "ps")
            pv_mm(7, pes.pop(7))
            rec(6)
            pv_norm(6)
            yp_pre(2, ps_a, "po")
            rec(7)
            pv_norm(7)
            yp_pre(3, ps_a, "po")
            for qt in range(NDC):
                yp_fin(qt)

    nc.finalize()
    return nc


_CACHE = {}


def _get_nc():
    if "nc" not in _CACHE:
        _CACHE["nc"] = _build_nc()
    return _CACHE["nc"]


def _to_sub(a, dt):
    """[rows, D] -> [128, NDC, rows] (contraction-subtiled) in dtype dt."""
    x = np.ascontiguousarray(
        a.T.reshape(NDC, P, -1).transpose(1, 0, 2)).astype(dt)
    return np.ascontiguousarray(x).reshape(P, -1)


def _prep_shared(W1, b1, W2, b2, W3, b3, Wo, bo):
    f = np.float32
    W_o_ = Wo
    shared = {
        "w1d": _to_sub(np.asarray(W1, f) * f(16.0), FP8),
        "w2d": _to_sub(np.asarray(W2, f) * f(16.0), FP8),
        "w3d": _to_sub(np.asarray(W3, f), BF16),
        "wod": _to_sub(np.asarray(Wo, f), BF16),
        "b1d": np.ascontiguousarray((np.asarray(b1, f) * f(0.5))
                                    .reshape(NDC, P).T),
        "bod": np.broadcast_to(
            np.asarray(bo, f) + np.asarray(W_o_, f) @ np.asarray(b3, f),
            (P, D)).copy(),
    }
    return shared


def build_in_maps(q_in, k_in, v_in, mask, W1, b1, W2, b2, W3, b3, Wo, bo):
    f = np.float32
    q_in = np.asarray(q_in, f)
    k_in = np.asarray(k_in, f)
    v_in = np.asarray(v_in, f)
    mask = np.asarray(mask)
    shared = _prep_shared(W1, b1, W2, b2, W3, b3, Wo, bo)
    xkd = [_to_sub(k_in[b], FP8) for b in range(B)]
    # xv layout [c, kb, s, key%1024]
    xvd = []
    for b in range(B):
        xs = _to_sub(v_in[b], BF16).reshape(P, NDC, 2, 1024)
        xvd.append(np.ascontiguousarray(
            xs.transpose(0, 2, 1, 3)).reshape(P, NDC * S))
    in_maps = []
    for c in range(NCORES):
        b, r = divmod(c, RPB)
        sl = slice(r * QB, (r + 1) * QB)
        # mask [c, g, j, q] fp16 in {0, 1}
        mT = mask[b, 0, sl, :].T  # [key, q]
        mkk = np.ascontiguousarray(
            mT.reshape(NKT, P, QB).transpose(1, 0, 2)).astype(np.float16)
        in_maps.append({
            "xqd": _to_sub(q_in[b, sl, :], FP8),
            "xkd": xkd[b],
            "xvd": xvd[b],
            "mkd": mkk.reshape(P, NKT * QB),
            **shared,
        })
    return in_maps


def kernel(q_in, k_in, v_in, mask, W1, b1, W2, b2, W3, b3, Wo, bo):
    f = np.float32
    nc = _get_nc()
    in_maps = build_in_maps(q_in, k_in, v_in, mask, W1, b1, W2, b2, W3, b3,
                            Wo, bo)
    res = run_bass_kernel_spmd(nc, in_maps, list(range(NCORES)))
    out = np.empty((B, S, D), f)
    for c in range(NCORES):
        b, r = divmod(c, RPB)
        out[b, r * QB:(r + 1) * QB, :] = res.results[c]["y"]
    return out


# revision 20
# speedup vs baseline: 1.0503x; 1.0503x over previous
"""Multi-head attention (dense transformer block) on 8 Trainium2 NeuronCores.

Sharding: batch (2) x query-row-block (4) -> 8 cores; each core computes the
full attention output for its 512 query rows of one batch.

v3 pipeline (per core):
  - Q/K projections: fp8e4 DoubleRow matmuls (0.5 cyc/row), outputs stored
    fp8 in head-pair tiles (heads 2m / 2m+1 on partition halves).
  - scores: fp8 DoubleRow with a shared zero block-1 (custom-stride APs), so
    the 64-deep head contraction runs at 0.5 cyc/row.
  - exp on ACT (psum f32 -> fp16, scale folds the 1/sqrt(HD)); the 0/1 mask
    is applied as an in-place fp16 DVE multiply (exact zeros).
  - V projection in bf16 (fp8 here dominates the error budget); V stored
    fp16 with a ones column for softmax denominators.
  - p@[V|1] in fp16; denominators -> reciprocal -> broadcast -> scale.
  - output projection bf16.
  - K bias is dropped entirely: it shifts scores per-query only and cancels
    in softmax.
Scale bookkeeping: W1/W2 sent x16 (fp8 subnormal headroom), rescaled at PSUM
evacuation; q kept at 4x the 1/8-scaled value, Exp applies scale=0.25.
"""

import numpy as np
import ml_dtypes
import bass_rust
import concourse.bass as bass
import concourse.mybir as mybir
from concourse import bacc
from concourse.tile import TileContext
from concourse.bass_utils import run_bass_kernel_spmd

B, S, D, H, HD = 2, 2048, 512, 8, 64
P = 128
NCORES = 8
RPB = 4              # q-row blocks per batch
QB = S // RPB        # 512 query rows per core
NDC = D // P         # 4 chunks of the model dim (contraction subtiles)
NKT = S // P         # 16 key tiles
NG = NKT // 2        # 8 key-tile pairs (one scores psum tile each)
VW = HD + 1          # 65 columns per head in vext (64 v + 1 ones)
KZ = NKT * P         # zero-block column offset in kp tiles
QZ = QB              # zero-block column offset in qp tiles

f32 = mybir.dt.float32
f32r = mybir.dt.float32r
f16 = mybir.dt.float16
f8 = mybir.dt.float8e4
bf16 = mybir.dt.bfloat16
u32 = mybir.dt.uint32
Alu = mybir.AluOpType
ActF = mybir.ActivationFunctionType
DR = mybir.MatmulPerfMode.DoubleRow

FP8 = ml_dtypes.float8_e4m3fn
BF16 = ml_dtypes.bfloat16


def _dr_ap(tile, prow, nrow, dims, coloff):
    """Custom strided AP into `tile` starting at partition `prow`, column
    `coloff`, with explicit free dims `dims` ([[stride, count], ...])."""
    a = tile[:].copy()
    rs = a.ap[0][0]
    a.ap = bass_rust.VecI64Pair([[rs, nrow]] + dims)
    a.offset = prow * rs + coloff
    return a


def _build_nc(repeats=1):
    nc = bacc.Bacc("TRN2", target_bir_lowering=False, debug=False,
                   num_devices=NCORES)
    xqd = nc.dram_tensor("xqd", [P, NDC * QB], f8, kind="ExternalInput")
    xkd = nc.dram_tensor("xkd", [P, NDC * S], f8, kind="ExternalInput")
    # xv layout: [c, kb(2), s(4), key%1024] for key-range-split DMA
    xvd = nc.dram_tensor("xvd", [P, NDC * S], bf16, kind="ExternalInput")
    w1d = nc.dram_tensor("w1d", [P, NDC * D], f8, kind="ExternalInput")
    w2d = nc.dram_tensor("w2d", [P, NDC * D], f8, kind="ExternalInput")
    w3d = nc.dram_tensor("w3d", [P, NDC * D], bf16, kind="ExternalInput")
    wod = nc.dram_tensor("wod", [P, NDC * D], bf16, kind="ExternalInput")
    # mask [c, g(8), j(2), q] in {0,1} fp16
    mkd = nc.dram_tensor("mkd", [P, NKT * QB], f16, kind="ExternalInput")
    b1d = nc.dram_tensor("b1d", [P, NDC], f32, kind="ExternalInput")
    bod = nc.dram_tensor("bod", [P, D], f32, kind="ExternalInput")
    y = nc.dram_tensor("y", [QB, D], f32, kind="ExternalOutput")

    with TileContext(nc) as tc, nc.allow_low_precision("fp8 attention"):
        with (
            tc.tile_pool(name="persist", bufs=1) as pp,
            tc.tile_pool(name="small", bufs=1) as sp,
            tc.tile_pool(name="psp", bufs=2, space="PSUM") as psp,
            tc.tile_pool(name="ps_s", bufs=2, space="PSUM") as ps_s,
            tc.tile_pool(name="ps_a", bufs=2, space="PSUM") as ps_a,
            tc.tile_pool(name="pex", bufs=18) as pex,
            tc.tile_pool(name="asm", bufs=2) as asm,
            tc.tile_pool(name="yp", bufs=4) as yp,
        ):
            # ---- persistent SBUF tiles ----
            # kp[m]: heads 2m (part 0-63) / 2m+1 (64-127); cols t*128+k,
            # shared zero block at [*, KZ:KZ+128]
            kp = [pp.tile([P, KZ + P], f8, tag=f"kp{m}", name=f"kp{m}")
                  for m in range(NDC)]
            # qp[m]: cols 0..511 = q data, 512..767 shared zero
            qp = [pp.tile([P, QB + 256], f8, tag=f"qp{m}", name=f"qp{m}")
                  for m in range(NDC)]
            # vext[g]: [128, h*2*VW + j*VW + e] fp16, ones at e=64
            vext = [pp.tile([P, H * 2 * VW], f16, tag=f"vx{g}",
                            name=f"vx{g}") for g in range(NG)]
            mk = pp.tile([P, NKT * QB], f16, tag="mk", name="mk")
            wosb = pp.tile([P, NDC * D], bf16, tag="wosb", name="wosb")
            outT = [pp.tile([P, QB], bf16, tag=f"outT{m}", name=f"outT{m}")
                    for m in range(NDC)]

            xq = sp.tile([P, NDC * QB], f8, tag="xq", name="xq")
            xk = sp.tile([P, NDC * S], f8, tag="xk", name="xk")
            xv = sp.tile([P, NDC * S], bf16, tag="xv", name="xv")
            w1 = sp.tile([P, NDC * D], f8, tag="w1", name="w1")
            w2 = sp.tile([P, NDC * D], f8, tag="w2", name="w2")
            w3 = sp.tile([P, NDC * D], bf16, tag="w3", name="w3")
            b1c = sp.tile([P, NDC], f32, tag="b1c", name="b1c")
            bob = sp.tile([P, D], f32, tag="bob", name="bob")
            ones_r = sp.tile([1, P], f32r, tag="ones_r", name="ones_r")
            nc.vector._memset_packed(ones_r[:].bitcast(u32), 0x3F800000)

            # ---- DMA (two rings: sync=SP, gpsimd=Pool) ----
            HX = NDC * S // 2
            nc.gpsimd.dma_start(w1[:], w1d[:])
            nc.gpsimd.dma_start(xk[:, HX:], xkd[:, HX:])
            nc.gpsimd.dma_start(w2[:], w2d[:])

            # zero blocks for the DoubleRow score trick; V ones columns
            for m in range(NDC):
                nc.gpsimd.memset(kp[m][:, KZ:KZ + P], 0.0)
                nc.gpsimd.memset(qp[m][:, QZ:QZ + 256], 0.0)
            for g in range(NG):
                vv = vext[g][:].rearrange("p (h j e) -> p h j e", j=2, e=VW)
                nc.gpsimd.memset(vv[:, :, :, HD:VW], 1.0)

            nc.gpsimd.dma_start(w3[:], w3d[:])
            HM = NKT * QB // 2
            nc.sync.dma_start(xq[:], xqd[:])
            nc.sync.dma_start(b1c[:], b1d[:])
            nc.sync.dma_start(xk[:, 0:HX], xkd[:, 0:HX])
            nc.sync.dma_start(xv[:, 0:HX], xvd[:, 0:HX])
            nc.sync.dma_start(mk[:, 0:HM], mkd[:, 0:HM])
            nc.sync.dma_start(xv[:, HX:], xvd[:, HX:])
            nc.sync.dma_start(mk[:, HM:], mkd[:, HM:])
            nc.sync.dma_start(bob[:], bod[:])
            nc.sync.dma_start(wosb[:], wod[:])

            xqv = xq[:].rearrange("p (s q) -> p s q", s=NDC)
            xkv = xk[:].rearrange("p (s k) -> p s k", s=NDC)
            # xv view: [c, kb, s, kk]
            xvv = xv[:].rearrange("p (kb s k) -> p kb s k", kb=2, s=NDC)
            w1v = w1[:].rearrange("p (s d) -> p s d", s=NDC)
            w2v = w2[:].rearrange("p (s d) -> p s d", s=NDC)
            w3v = w3[:].rearrange("p (s d) -> p s d", s=NDC)

            def dr_pair_mm(ps_region, stat3, mov3):
                """512-deep contraction (2 DoubleRow pair-calls) into a
                256-col psum region; completes the group start..stop."""
                for pr in range(2):
                    nc.tensor.matmul(ps_region, stat3(pr), mov3(pr),
                                     start=(pr == 0), stop=(pr == 1),
                                     perf_mode=DR)

            # ==== Q projection ====
            def q_proj(ms):
                for m in ms:
                    ps = psp.tile([P, QB], f32, tag="ps", name="psq")
                    for qh in range(2):
                        dr_pair_mm(
                            ps[:, qh * 256:(qh + 1) * 256],
                            lambda pr: w1v[:, 2 * pr:2 * pr + 2,
                                           m * P:(m + 1) * P],
                            lambda pr: xqv[:, 2 * pr:2 * pr + 2,
                                           qh * 256:(qh + 1) * 256])
                    nc.vector.tensor_scalar(
                        qp[m][:, 0:QB], ps[:],
                        1.0 / 32.0, b1c[:, m:m + 1], Alu.mult, Alu.add)

            # ==== K projection (no bias: cancels in softmax) ====
            # evacs for m>=1 run on Pool: the DVE queue is ACT-gated by the
            # mask multiplies, which would delay these (and the psum ring)
            # by a full exp period per head.
            def k_proj(m):
                for kb in range(NDC):
                    ps = psp.tile([P, QB], f32, tag="ps", name="psk")
                    for qh in range(2):
                        c0 = kb * QB + qh * 256
                        dr_pair_mm(
                            ps[:, qh * 256:(qh + 1) * 256],
                            lambda pr: w2v[:, 2 * pr:2 * pr + 2,
                                           m * P:(m + 1) * P],
                            lambda pr: xkv[:, 2 * pr:2 * pr + 2,
                                           c0:c0 + 256])
                    nc.vector.tensor_scalar(
                        kp[m][:, kb * QB:(kb + 1) * QB], ps[:],
                        1.0 / 16.0, None, Alu.mult)

            # ==== scores + exp + mask ====
            def scores_head(h, embeds=None):
                m, hp = divmod(h, 2)
                base = hp * HD
                pes = []
                for g in range(NG):
                    ss = ps_s.tile([P, 2 * QB], f32, tag="ss", name="ss")
                    for j in range(2):
                        t = 2 * g + j
                        st = _dr_ap(kp[m], base, HD,
                                    [[KZ - t * P, 2], [1, P]], t * P)
                        for qh in range(2):
                            mv = _dr_ap(qp[m], base, HD,
                                        [[QZ - qh * 256, 2], [1, 256]],
                                        qh * 256)
                            nc.tensor.matmul(
                                ss[:, j * QB + qh * 256:
                                   j * QB + (qh + 1) * 256],
                                st, mv, start=True, stop=True, perf_mode=DR)
                    pe = pex.tile([P, 2 * QB], f16, tag="pe", name="pe",
                                  bufs=26)
                    nc.scalar.activation(pe[:], ss[:], ActF.Exp, scale=0.25)
                    # mask: in-place multiply by {0,1} fp16. Late heads go to
                    # Pool so tail reciprocals are not stuck behind the
                    # ACT-gated mult stream in the in-order DVE queue.
                    meng = nc.gpsimd
                    meng.tensor_tensor(
                        pe[:], pe[:], mk[:, g * 2 * QB:(g + 1) * 2 * QB],
                        op=Alu.mult)
                    pes.append(pe)
                    if embeds and g in embeds:
                        embeds[g]()  # e.g. a reciprocal, slotted into the
                        #              ACT-gated gaps of the DVE stream
                return pes

            # ==== V projection (bf16) ====
            def v_proj(t0, t1):
                for t in range(t0, t1):
                    kb, kk = divmod(t, 8)
                    pool = psp if t % 2 == 0 else ps_a
                    ps = pool.tile([P, D], f32, tag="ps" if t % 2 == 0
                                   else "po", name="psv")
                    for s in range(NDC):
                        nc.tensor.matmul(
                            ps[:],
                            xvv[:, kb, s, kk * P:(kk + 1) * P],
                            w3v[:, s, :],
                            start=(s == 0), stop=(s == NDC - 1))
                    g, j = divmod(t, 2)
                    vv = vext[g][:].rearrange("p (h j e) -> p h j e",
                                              j=2, e=VW)
                    nc.vector.tensor_copy(
                        vv[:, :, j:j + 1, 0:HD],
                        ps[:].rearrange("p (h o e) -> p h o e", o=1, e=HD))

            rhs = {}
            pos = {}

            def pv_mm(h, pes):
                """p @ [V|1] for head h (fp16). Pools alternate by head so
                each pool's 2-deep ring gives an effective lag of 4 heads."""
                pool, tag = (psp, "ps") if h % 2 == 0 else (ps_a, "po")
                po = pool.tile([VW, QB], f32, tag=tag, name="po")
                for t in range(NKT):
                    g, j = divmod(t, 2)
                    nc.tensor.matmul(
                        po[:],
                        vext[g][:, h * 2 * VW + j * VW:
                                h * 2 * VW + (j + 1) * VW],
                        pes[g][:, j * QB:(j + 1) * QB],
                        start=(t == 0), stop=(t == NKT - 1))
                pos[h] = po

            def rec(h):
                """reciprocal of head h's softmax denominator (DVE)."""
                rh = asm.tile([1, QB], f32, tag="rh", name="rh", bufs=2)
                nc.vector.reciprocal(rh[:], pos[h][HD:VW, :])
                rhs[h] = rh

            rbs = {}

            def bcast(h):
                rb = asm.tile([HD, QB], f32, tag="rb", name="rb", bufs=3)
                nc.gpsimd.partition_broadcast(rb[:], rhs.pop(h)[:])
                rbs[h] = rb

            def outm(h):
                m, hp = divmod(h, 2)
                nc.vector.tensor_tensor(
                    outT[m][hp * HD:(hp + 1) * HD, :],
                    pos.pop(h)[0:HD, :], rbs.pop(h)[:], op=Alu.mult)

            # ---- emission schedule (ACT = bottleneck; keep it fed).
            # Reciprocals are embedded into later heads' scores streams so
            # the in-order DVE reaches them in the ACT-gated gaps between
            # mask multiplies.
            q_proj([0])
            k_proj(0)
            pes = {0: scores_head(0)}
            q_proj([1, 2, 3])
            k_proj(1)
            v_proj(0, 4)
            pes[1] = scores_head(1)
            v_proj(4, 8)
            k_proj(2)
            pes[2] = scores_head(2)
            v_proj(8, 16)
            k_proj(3)
            pes[3] = scores_head(3)
            pv_mm(0, pes.pop(0))
            pes[4] = scores_head(4, {0: lambda: rec(0),
                                     4: lambda: bcast(0)})
            pv_mm(1, pes.pop(1))
            outm(0)
            pes[5] = scores_head(5, {0: lambda: rec(1),
                                     4: lambda: bcast(1)})
            pv_mm(2, pes.pop(2))
            outm(1)
            pes[6] = scores_head(6, {0: lambda: rec(2),
                                     4: lambda: bcast(2)})
            pv_mm(3, pes.pop(3))
            outm(2)
            pv_mm(4, pes.pop(4))
            rec(3)
            pes[7] = scores_head(7, {2: lambda: bcast(3),
                                     5: lambda: rec(4)})
            outm(3)
            pv_mm(5, pes.pop(5))
            rec(5)
            pv_mm(6, pes.pop(6))
            rec(6)
            pv_mm(7, pes.pop(7))
            rec(7)
            bcast(4)
            outm(4)
            bcast(5)
            outm(5)
            bcast(6)
            outm(6)
            bcast(7)
            outm(7)

            # ==== output projection (tail; psums ride the dead ss ring) ====
            yps = {}
            for qt in (0, 1):
                ps = ps_s.tile([P, D], f32, tag="ss", name="psy")
                for m in range(3):
                    nc.tensor.matmul(
                        ps[:], outT[m][:, qt * P:(qt + 1) * P],
                        wosb[:, m * D:(m + 1) * D],
                        start=(m == 0), stop=False)
                yps[qt] = ps
            for qt in range(NDC):
                if qt in yps:
                    ps = yps[qt]
                    nc.tensor.matmul(
                        ps[:], outT[3][:, qt * P:(qt + 1) * P],
                        wosb[:, 3 * D:4 * D], start=False, stop=True)
                else:
                    ps = ps_s.tile([P, D], f32, tag="ss", name="psy")
                    for m in range(NDC):
                        nc.tensor.matmul(
                            ps[:],
                            outT[m][:, qt * P:(qt + 1) * P],
                            wosb[:, m * D:(m + 1) * D],
                            start=(m == 0), stop=(m == NDC - 1))
                ysb = yp.tile([P, D], f32, tag="ysb", name="ysb", bufs=4)
                nc.vector.tensor_tensor(ysb[:], ps[:], bob[:], op=Alu.add)
                nc.sync.dma_start(y[qt * P:(qt + 1) * P, :], ysb[:])

    nc.finalize()
    return nc


_CACHE = {}


def _get_nc():
    if "nc" not in _CACHE:
        _CACHE["nc"] = _build_nc()
    return _CACHE["nc"]


def _to_sub(a, dt):
    """[rows, D] -> [128, NDC, rows] (contraction-subtiled) in dtype dt."""
    x = np.ascontiguousarray(
        a.T.reshape(NDC, P, -1).transpose(1, 0, 2)).astype(dt)
    return np.ascontiguousarray(x).reshape(P, -1)


def _prep_shared(W1, b1, W2, b2, W3, b3, Wo, bo):
    f = np.float32
    W_o_ = Wo
    shared = {
        "w1d": _to_sub(np.asarray(W1, f) * f(16.0), FP8),
        "w2d": _to_sub(np.asarray(W2, f) * f(16.0), FP8),
        "w3d": _to_sub(np.asarray(W3, f), BF16),
        "wod": _to_sub(np.asarray(Wo, f), BF16),
        "b1d": np.ascontiguousarray((np.asarray(b1, f) * f(0.5))
                                    .reshape(NDC, P).T),
        "bod": np.broadcast_to(
            np.asarray(bo, f) + np.asarray(W_o_, f) @ np.asarray(b3, f),
            (P, D)).copy(),
    }
    return shared


def build_in_maps(q_in, k_in, v_in, mask, W1, b1, W2, b2, W3, b3, Wo, bo):
    f = np.float32
    q_in = np.asarray(q_in, f)
    k_in = np.asarray(k_in, f)
    v_in = np.asarray(v_in, f)
    mask = np.asarray(mask)
    shared = _prep_shared(W1, b1, W2, b2, W3, b3, Wo, bo)
    xkd = [_to_sub(k_in[b], FP8) for b in range(B)]
    # xv layout [c, kb, s, key%1024]
    xvd = []
    for b in range(B):
        xs = _to_sub(v_in[b], BF16).reshape(P, NDC, 2, 1024)
        xvd.append(np.ascontiguousarray(
            xs.transpose(0, 2, 1, 3)).reshape(P, NDC * S))
    in_maps = []
    for c in range(NCORES):
        b, r = divmod(c, RPB)
        sl = slice(r * QB, (r + 1) * QB)
        # mask [c, g, j, q] fp16 in {0, 1}
        mT = mask[b, 0, sl, :].T  # [key, q]
        mkk = np.ascontiguousarray(
            mT.reshape(NKT, P, QB).transpose(1, 0, 2)).astype(np.float16)
        in_maps.append({
            "xqd": _to_sub(q_in[b, sl, :], FP8),
            "xkd": xkd[b],
            "xvd": xvd[b],
            "mkd": mkk.reshape(P, NKT * QB),
            **shared,
        })
    return in_maps


def kernel(q_in, k_in, v_in, mask, W1, b1, W2, b2, W3, b3, Wo, bo):
    f = np.float32
    nc = _get_nc()
    in_maps = build_in_maps(q_in, k_in, v_in, mask, W1, b1, W2, b2, W3, b3,
                            Wo, bo)
    res = run_bass_kernel_spmd(nc, in_maps, list(range(NCORES)))
    out = np.empty((B, S, D), f)
    for c in range(NCORES):
        b, r = divmod(c, RPB)
        out[b, r * QB:(r + 1) * QB, :] = res.results[c]["y"]
    return out
